# revision 30
# baseline (speedup 1.0000x reference)
"""DeepSeek sparse attention on 8 Trainium2 NeuronCores.

Head-sharded (2 heads/core). Per core:
  - indexer scores X = iq @ ik^T via bf16 hi/lo 3-pass matmul (fp32-grade
    precision at 3 cyc/row vs fp32's 4)
  - top-32 threshold per query: top-8 of 16 stride-subsets (DVE InstMax)
    -> 4 rounds max8+match_replace -> t32 = 32nd-largest candidate
  - mask M = (X >= t32): Sign on Scalar engine, is_ge on GpSimd,
    DMA-transposed to [s,q]
  - attention in [s,q]: E^T = exp(S^T) (Scalar), W^T = E^T * M^T (GpSimd),
    AV via ones-augmented V (free denominators), normalize in [q,dh]
  - issue order overlaps head-1 selection (DVE-bound) with head-0
    attention (PE/Act/GpSimd-bound); out_proj partial per core; host sums.
"""
import sys

sys.path.insert(0, '/opt/trn_rl_repo')
sys.path.insert(0, '/opt/pypackages')

import numpy as np
import ml_dtypes

BF16 = ml_dtypes.bfloat16

B, T, D = 1, 2048, 1024
H, DH, DI, KSEL = 16, 64, 32, 32
NCORES = 8
HPC = H // NCORES          # heads per core
NT = T // 128              # 16 query/key tiles
NK = D // 128              # 8 contraction chunks

_COMPILED = {}

# "hilo": 3-pass bf16 X matmul (fp32-grade, rel~0.009, 3 cyc/row)
# "fp32r": single-pass fp32r X matmul (rel~0.016, 1 cyc/row)
X_MODE = "fp32r"


def _install_drain_patch():
    import concourse.mybir as mybir
    from concourse.tile import TileContext
    from concourse.vector_clock import ScopedClock

    if getattr(TileContext, "_dsa_patched", False):
        return

    def _patched(self, tick_clock, wait_clock):
        nc = self.nc
        drain_inst = nc.sync.drain()
        wait_clock.add_sem_waits(
            drain_inst.ins, ScopedClock({None: tick_clock.global_clock})
        )
        si = drain_inst.ins.sync_info
        waits = list(si.on_wait or []) if si is not None else []
        if len(waits) > 1:
            drain_inst.ins.sync_info = mybir.SyncInfo(
                on_wait=waits[:1], on_update=list(si.on_update or [])
            )
            for i in range(1, len(waits)):
                extra = nc.sync.drain()
                extra.ins.sync_info = mybir.SyncInfo(
                    on_wait=waits[i:i + 1], on_update=[]
                )
        nc.all_engine_barrier()
        assert self.sems is not None
        popped = nc._tile_sem_poison_stack.pop()
        assert popped is self._sem_poison
        nc.clear_and_free_semaphores(list(self.sems.allocated().values()))
        nc.all_engine_barrier()

    TileContext._drain_and_barrier = _patched
    TileContext._dsa_patched = True


def _split_excess_waits(nc, limit=1):
    """walrus in this container rejects instructions with more sync waits
    than the ISA struct encodes; hoist excess waits onto standalone
    EventSemaphore instructions on the same engine, inserted just before."""
    import concourse.mybir as mybir

    n_new = 0
    for bb in nc.main_func.blocks:
        insts = bb.instructions
        i = 0
        while i < len(insts):
            ins = insts[i]
            si = ins.sync_info
            waits = list(si.on_wait or []) if si is not None else []
            if len(waits) > limit:
                ins.sync_info = mybir.SyncInfo(
                    on_wait=waits[:limit], on_update=list(si.on_update or []))
                pos = i
                for j in range(limit, len(waits), limit):
                    n_new += 1
                    w = mybir.InstEventSemaphore(
                        name=f"WSPLIT-{n_new}", ins=[], outs=[])
                    w.engine = ins.engine
                    w.sync_info = mybir.SyncInfo(
                        on_wait=waits[j:j + limit], on_update=[])
                    nc.register_instruction(w, overwrite=True)
                    insts.insert(pos, w)
                    pos += 1
                    i += 1
            i += 1
    return n_new


def _build_module():
    import concourse.bass as bass
    import concourse.mybir as mybir
    from concourse.tile import TileContext

    _install_drain_patch()
    dt = mybir.dt
    nc = bass.Bass()

    hsT_hi = nc.declare_dram_parameter("hsT_hi", [D, T], dt.bfloat16, isOutput=False)
    hsT_lo = nc.declare_dram_parameter("hsT_lo", [D, T], dt.bfloat16, isOutput=False)
    A_hi = nc.declare_dram_parameter("A_hi", [D, 128], dt.bfloat16, isOutput=False)
    A_lo = nc.declare_dram_parameter("A_lo", [D, 128], dt.bfloat16, isOutput=False)
    Wqk_h0 = nc.declare_dram_parameter("Wqk_h0", [D, 128], dt.bfloat16, isOutput=False)
    Wqk_h1 = nc.declare_dram_parameter("Wqk_h1", [D, 128], dt.bfloat16, isOutput=False)
    Wv_cat = nc.declare_dram_parameter("Wv_cat", [D, 128], dt.bfloat16, isOutput=False)
    WoT_cat = nc.declare_dram_parameter("WoT_cat", [128, D], dt.bfloat16, isOutput=False)
    out_part = nc.declare_dram_parameter("out_part", [T, D], dt.float32, isOutput=True)

    Sigmoid = mybir.ActivationFunctionType.Sigmoid
    Exp = mybir.ActivationFunctionType.Exp
    MUL = mybir.AluOpType.mult
    GE = mybir.AluOpType.is_ge
    SUB = mybir.AluOpType.subtract

    with TileContext(nc) as tc:
        with tc.tile_pool(name="state", bufs=1) as st:
            if X_MODE == "fp32r":
                IqR = st.tile([64, T], dt.float32r, tag="IqR")
                IkR = st.tile([64, T], dt.float32r, tag="IkR")
            else:
                Iqh = st.tile([64, T], dt.bfloat16, tag="Iqh")
                Iql = st.tile([64, T], dt.bfloat16, tag="Iql")
                Ikh = st.tile([64, T], dt.bfloat16, tag="Ikh")
                Ikl = st.tile([64, T], dt.bfloat16, tag="Ikl")
            QT = st.tile([128, T], dt.bfloat16, tag="QT")
            KT = st.tile([128, T], dt.bfloat16, tag="KT")
            VP = st.tile([128, NT, 2, 65], dt.bfloat16, tag="VP")
            ATcatT = st.tile([128, T], dt.bfloat16, tag="ATcatT")
            wo = st.tile([128, D], dt.bfloat16, tag="wo")
            nc.sync.dma_start(out=wo[:], in_=WoT_cat[:])

            # ================= P0: projections =================
            with tc.tile_pool(name="hsbp", bufs=1) as hp, \
                 tc.tile_pool(name="p0w", bufs=1) as p0w, \
                 tc.tile_pool(name="p0p", bufs=1, space="PSUM") as p0p:
                hsb = hp.tile([128, NK, T], dt.bfloat16, tag="hsb")
                hslo = hp.tile([128, NK, T], dt.bfloat16, tag="hslo")
                a_h = p0w.tile([128, NK, 128], dt.bfloat16, tag="a_h")
                a_l = p0w.tile([128, NK, 128], dt.bfloat16, tag="a_l")
                qk0_w = p0w.tile([128, NK, 128], dt.bfloat16, tag="qk0_w")
                qk1_w = p0w.tile([128, NK, 128], dt.bfloat16, tag="qk1_w")
                v_w = p0w.tile([128, NK, 128], dt.bfloat16, tag="v_w")
                nc.sync.dma_start(out=a_h[:], in_=A_hi[:].rearrange("(c p) m -> p c m", p=128))
                nc.sync.dma_start(out=a_l[:], in_=A_lo[:].rearrange("(c p) m -> p c m", p=128))
                nc.sync.dma_start(out=qk0_w[:], in_=Wqk_h0[:].rearrange("(c p) m -> p c m", p=128))
                nc.sync.dma_start(out=qk1_w[:], in_=Wqk_h1[:].rearrange("(c p) m -> p c m", p=128))
                nc.sync.dma_start(out=v_w[:], in_=Wv_cat[:].rearrange("(c p) m -> p c m", p=128))
                for k in range(NK):
                    nc.sync.dma_start(out=hsb[:, k, :], in_=hsT_hi[128 * k:128 * k + 128, :])
                for k in range(NK):
                    nc.sync.dma_start(out=hslo[:, k, :], in_=hsT_lo[128 * k:128 * k + 128, :])

                ip = p0p.tile([128, T], dt.float32, tag="ip")
                qp = p0p.tile([128, T], dt.float32, tag="qp")
                # pass A: I (hi/lo 3-pass) + QK_h0 (bf16)
                for k in range(NK):
                    for n in range(4):
                        sl = slice(512 * n, 512 * n + 512)
                        nc.tensor.matmul(ip[:, sl], a_h[:, k, :], hsb[:, k, sl],
                                         start=(k == 0), stop=False)
                        nc.tensor.matmul(ip[:, sl], a_l[:, k, :], hsb[:, k, sl],
                                         start=False, stop=False)
                        nc.tensor.matmul(ip[:, sl], a_h[:, k, :], hslo[:, k, sl],
                                         start=False, stop=(k == NK - 1))
                    for n in range(4):
                        sl = slice(512 * n, 512 * n + 512)
                        nc.tensor.matmul(qp[:, sl], qk0_w[:, k, :], hsb[:, k, sl],
                                         start=(k == 0), stop=(k == NK - 1))
                # evac indexer projections
                if X_MODE == "fp32r":
                    nc.vector.tensor_copy(IqR[:], ip[0:64, :])
                    nc.vector.tensor_copy(IkR[:], ip[64:128, :])
                else:
                    nc.scalar.copy(out=Iqh[:], in_=ip[0:64, :])
                    nc.scalar.copy(out=Ikh[:], in_=ip[64:128, :])
                    nc.vector.tensor_tensor(out=Iql[:], in0=ip[0:64, :], in1=Iqh[:], op=SUB)
                    nc.vector.tensor_tensor(out=Ikl[:], in0=ip[64:128, :], in1=Ikh[:], op=SUB)
                nc.scalar.copy(out=QT[0:64, :], in_=qp[0:64, :])
                nc.scalar.copy(out=KT[0:64, :], in_=qp[64:128, :])

                # pass B: QK_h1 + V (bf16)
                qp1 = p0p.tile([128, T], dt.float32, tag="ip")  # reuse slot
                vp_ps = p0p.tile([128, T], dt.float32, tag="qp")
                for k in range(NK):
                    for n in range(4):
                        sl = slice(512 * n, 512 * n + 512)
                        nc.tensor.matmul(qp1[:, sl], qk1_w[:, k, :], hsb[:, k, sl],
                                         start=(k == 0), stop=(k == NK - 1))
                    for n in range(4):
                        sl = slice(512 * n, 512 * n + 512)
                        nc.tensor.matmul(vp_ps[:, sl], v_w[:, k, :], hsb[:, k, sl],
                                         start=(k == 0), stop=(k == NK - 1))
                nc.scalar.copy(out=QT[64:128, :], in_=qp1[0:64, :])
                nc.scalar.copy(out=KT[64:128, :], in_=qp1[64:128, :])
                # V: rows 0:64 = V_h0^T [dh, s], 64:128 = V_h1^T; DMA-transpose
                vt_b = p0w.tile([128, T], dt.bfloat16, tag="vt_b")
                nc.scalar.copy(out=vt_b[:], in_=vp_ps[:])
                vq = p0w.tile([128, NT, 128], dt.bfloat16, tag="vq")
                nc.sync.dma_start_transpose(out=vq[:], in_=vt_b[:])
                for j in range(NT):
                    for h in range(2):
                        nc.vector.tensor_copy(VP[:, j, h, 0:64], vq[:, j, 64 * h:64 * h + 64])
                        nc.vector.memset(VP[:, j, h, 64:65], 1.0)

            # ================= main phases =================
            with tc.tile_pool(name="mtp", bufs=2) as mtp, \
                 tc.tile_pool(name="pa", bufs=2) as pa, \
                 tc.tile_pool(name="pb", bufs=2) as pb, \
                 tc.tile_pool(name="pc", bufs=1) as pc, \
                 tc.tile_pool(name="pxp", bufs=2, space="PSUM") as pxp, \
                 tc.tile_pool(name="psp", bufs=1, space="PSUM") as psp, \
                 tc.tile_pool(name="pav", bufs=1, space="PSUM") as pav:

                mts = [mtp.tile([128, NT, NT, 128], dt.bfloat16, tag="mt",
                                name=f"mt{h}") for h in range(2)]
                ats = [pc.tile([128, T], dt.bfloat16, tag=f"at{h}", name=f"at{h}")
                       for h in range(2)]

                def emit_A_tile(h, i):
                    """selection for q-tile i of head h -> mask row in mts[h]"""
                    mt = mts[h]
                    cand = pa.tile([128, 128], dt.float32, tag="cand")
                    xps = []
                    for half in range(2):
                        xp = pxp.tile([128, 1024], dt.float32, tag="xp")
                        xps.append(xp)
                        for n in range(2):
                            sl = slice(512 * n, 512 * n + 512)
                            c0 = 1024 * half + 512 * n
                            qsl = slice(128 * i, 128 * i + 128)
                            if X_MODE == "fp32r":
                                nc.tensor.matmul(xp[:, sl],
                                                 IqR[32 * h:32 * h + 32, qsl],
                                                 IkR[32 * h:32 * h + 32, c0:c0 + 512])
                            else:
                                nc.tensor.matmul(xp[:, sl], Iqh[32 * h:32 * h + 32, qsl],
                                                 Ikh[32 * h:32 * h + 32, c0:c0 + 512],
                                                 start=True, stop=False)
                                nc.tensor.matmul(xp[:, sl], Iqh[32 * h:32 * h + 32, qsl],
                                                 Ikl[32 * h:32 * h + 32, c0:c0 + 512],
                                                 start=False, stop=False)
                                nc.tensor.matmul(xp[:, sl], Iql[32 * h:32 * h + 32, qsl],
                                                 Ikh[32 * h:32 * h + 32, c0:c0 + 512],
                                                 start=False, stop=True)
                        v8 = xp[:].rearrange("p (s l) -> p l s", l=8)
                        for j in range(8):
                            nc.vector.max(out=cand[:, 64 * half + 8 * j:64 * half + 8 * j + 8],
                                          in_=v8[:, j, :])
                    mx = pa.tile([128, 8], dt.float32, tag="mx")
                    for r in range(4):
                        nc.vector.max(out=mx[:], in_=cand[:])
                        if r < 3:
                            nc.vector.match_replace(out=cand[:], in_to_replace=mx[:],
                                                    in_values=cand[:], imm_value=-1e30)
                    # hard 0/1 mask via saturated sigmoid: 1e7*(X - t32 + 1e-5)
                    # puts the tie element at +100 (->1.0) and everything one
                    # ulp below threshold at <-100 (->0.0)
                    negt = pa.tile([128, 1], dt.float32, tag="negt")
                    nc.vector.tensor_scalar(negt[:], mx[:, 7:8], -1e7, scalar2=100.0,
                                            op0=MUL, op1=mybir.AluOpType.add)
                    ms = pa.tile([128, T], dt.bfloat16, tag="ms", bufs=1, name="ms")
                    for half in range(2):
                        nc.scalar.activation(out=ms[:, 1024 * half:1024 * half + 1024],
                                             in_=xps[half][:], func=Sigmoid,
                                             bias=negt[:], scale=1e7)
                    nc.sync.dma_start_transpose(out=mt[:, :, i, :], in_=ms[:])

                def emit_B_front(h, half, j):
                    """S^T matmul + exp for s-tile j, query half; returns e"""
                    e = pb.tile([128, 1024], dt.bfloat16, tag="e", bufs=5, name="e")
                    for n in range(2):
                        sp = psp.tile([128, 512], dt.float32, tag="sp", name="sp")
                        c0 = 1024 * half + 512 * n
                        nc.tensor.matmul(sp[:],
                                         KT[64 * h:64 * h + 64, 128 * j:128 * j + 128],
                                         QT[64 * h:64 * h + 64, c0:c0 + 512])
                        nc.scalar.activation(out=e[:, 512 * n:512 * n + 512],
                                             in_=sp[:], func=Exp)
                    return e

                bstate = {"av": None, "nback": 0}

                def emit_B_back(h, half, j, e, w_engine=None):
                    """mask-multiply + AV accumulate for a completed front"""
                    mt = mts[h]
                    if j == 0:
                        bstate["av"] = pav.tile([65, 1024], dt.float32, tag="av",
                                                name="av")
                    av = bstate["av"]
                    w = pb.tile([128, 1024], dt.bfloat16, tag="w", name="w")
                    msl = mt[:, j, 8 * half:8 * half + 8, :].rearrange("p a b -> p (a b)")
                    eng = w_engine if w_engine is not None else nc.gpsimd
                    eng.tensor_tensor(out=w[:], in0=e[:], in1=msl, op=MUL)
                    for n in range(2):
                        nc.tensor.matmul(av[:, 512 * n:512 * n + 512],
                                         VP[:, j, h, :], w[:, 512 * n:512 * n + 512],
                                         start=(j == 0), stop=(j == NT - 1))
                    if j == NT - 1:
                        nc.scalar.copy(out=ats[h][0:65, 1024 * half:1024 * half + 1024],
                                       in_=av[:])

                def emit_C(h):
                    """normalize + build transposed attn rows of ATcatT"""
                    at = ats[h]
                    atq = pc.tile([128, NT, 128], dt.bfloat16, tag="atq")
                    nc.sync.dma_start_transpose(out=atq[:], in_=at[:])
                    scrall = pc.tile([128, NT, 128], dt.bfloat16, tag="scrall")
                    rds = pa.tile([128, NT], dt.float32, tag="rds")
                    nc.vector.reciprocal(
                        rds[:], atq[:, :, 64:65].rearrange("p a b -> p (a b)"))
                    for i in range(NT):
                        nc.vector.tensor_scalar(scrall[:, i, 0:64], atq[:, i, 0:64],
                                                rds[:, i:i + 1], scalar2=None, op0=MUL)
                    tmpT = pc.tile([128, NT, 128], dt.bfloat16, tag="tmpT")
                    nc.sync.dma_start_transpose(out=tmpT[:], in_=scrall[:])
                    nc.vector.tensor_copy(ATcatT[64 * h:64 * h + 64, :],
                                          tmpT[0:64, :, :].rearrange("p a b -> p (a b)"))

                # ---- issue schedule ----
                LAG = 4  # (half,j) steps between a front and its back
                seq = [(half, j) for half in range(2) for j in range(NT)]

                # selection head 0
                for i in range(NT):
                    emit_A_tile(0, i)
                # head-1 selection overlapped with head-0 attention;
                # backs lag fronts so no engine blocks on another's latest op
                pend = []
                k = 0
                for i in range(NT):
                    emit_A_tile(1, i)
                    for _ in range(2):
                        half, j = seq[k]
                        k += 1
                        pend.append((half, j, emit_B_front(0, half, j)))
                        if len(pend) > LAG:
                            emit_B_back(0, *pend.pop(0))
                while pend:
                    emit_B_back(0, *pend.pop(0))
                emit_C(0)
                # head-1 attention tail; DVE is idle here, so alternate the
                # weight-multiply between gpsimd and vector
                nb = 0
                for half, j in seq:
                    pend.append((half, j, emit_B_front(1, half, j)))
                    if len(pend) > LAG:
                        eng = nc.vector if nb % 2 else nc.gpsimd
                        nb += 1
                        emit_B_back(1, *pend.pop(0), w_engine=eng)
                while pend:
                    eng = nc.vector if nb % 2 else nc.gpsimd
                    nb += 1
                    emit_B_back(1, *pend.pop(0), w_engine=eng)
                emit_C(1)

            # ================= out_proj =================
            with tc.tile_pool(name="po", bufs=2) as po, \
                 tc.tile_pool(name="pop", bufs=2, space="PSUM") as pop:
                for i in range(NT):
                    op = pop.tile([128, D], dt.float32, tag="op")
                    for n in range(2):
                        nc.tensor.matmul(op[:, 512 * n:512 * n + 512],
                                         ATcatT[:, 128 * i:128 * i + 128],
                                         wo[:, 512 * n:512 * n + 512])
                    ob = po.tile([128, D], dt.float32, tag="ob")
                    nc.scalar.copy(out=ob[:], in_=op[:])
                    nc.sync.dma_start(out=out_part[128 * i:128 * i + 128, :], in_=ob[:])

    _split_excess_waits(nc, limit=1)
    return nc


def _prep_inputs(hidden_states, Wq, Wk, Wv, Wo, idx_wq, idx_wk):
    hs = np.asarray(hidden_states[0], np.float32)          # [T, D]
    hsT = np.ascontiguousarray(hs.T)                       # [D, T]
    hsT_hi = hsT.astype(BF16)
    hsT_lo = (hsT - hsT_hi.astype(np.float32)).astype(BF16)
    maps = []
    for c in range(NCORES):
        h0, h1 = 2 * c, 2 * c + 1
        Aq_parts, Ak_parts = [], []
        for hh in (h0, h1):
            Wq_h = Wq[64 * hh:64 * hh + 64, :].astype(np.float64)    # [64, D]
            Wk_h = Wk[64 * hh:64 * hh + 64, :].astype(np.float64)
            Aq_parts.append((Wq_h.T @ idx_wq[hh].astype(np.float64)).astype(np.float32))
            Ak_parts.append((Wk_h.T @ idx_wk[hh].astype(np.float64)).astype(np.float32))
        A_cat = np.concatenate(Aq_parts + Ak_parts, axis=1)  # [D, 128]
        A_hi = A_cat.astype(BF16)
        A_lo = (A_cat - A_hi.astype(np.float32)).astype(BF16)

        def qk_chain(hh):
            Wq_h = Wq[64 * hh:64 * hh + 64, :]
            Wk_h = Wk[64 * hh:64 * hh + 64, :]
            return np.concatenate(
                [(Wq_h.T / np.sqrt(DH)).astype(BF16), Wk_h.T.astype(BF16)], axis=1)

        Wv_c = np.concatenate(
            [Wv[64 * h0:64 * h0 + 64, :].T, Wv[64 * h1:64 * h1 + 64, :].T],
            axis=1).astype(BF16)                           # [D, 128]
        WoT_c = np.ascontiguousarray(Wo[:, 64 * h0:64 * h0 + 128].T).astype(BF16)

        maps.append({
            "hsT_hi": hsT_hi,
            "hsT_lo": hsT_lo,
            "A_hi": A_hi,
            "A_lo": A_lo,
            "Wqk_h0": qk_chain(h0),
            "Wqk_h1": qk_chain(h1),
            "Wv_cat": Wv_c,
            "WoT_cat": WoT_c,
        })
    return maps


def kernel(hidden_states, Wq, Wk, Wv, Wo, idx_wq, idx_wk):
    from concourse.bass_utils import run_bass_kernel_spmd

    if "nc" not in _COMPILED:
        _COMPILED["nc"] = _build_module()
    nc = _COMPILED["nc"]

    in_maps = _prep_inputs(np.asarray(hidden_states), np.asarray(Wq),
                           np.asarray(Wk), np.asarray(Wv), np.asarray(Wo),
                           np.asarray(idx_wq), np.asarray(idx_wk))
    res = run_bass_kernel_spmd(nc, in_maps, core_ids=list(range(NCORES)))
    out = np.zeros((T, D), np.float32)
    for c in range(NCORES):
        out += np.asarray(res.results[c]["out_part"], np.float32)
    return out.reshape(B, T, D)
